# revision 30
# baseline (speedup 1.0000x reference)
"""Devign-GGNN Trainium2 kernel.

Full inputs in, full output out. Sharding: data-parallel over the B=32
graphs -> 4 graphs per NeuronCore on 8 cores. The gather/scatter message
passing is reformulated as dense per-(graph, etype) adjacency matmuls:

    a = sum_k A_k^T (h @ W_k) + bias_term,   A_k[s, d] = #edges(s->d, type k)

A (4 x 13 x 512 x 512 per core) is exact in fp8e4m3 (counts/8) and stays
resident in SBUF. ALL GGNN matmuls (stage-1 h@W_e, stage-2 A^T, GRU
gates) run fp8 x fp8 single-pass with the DoubleRow perf mode (0.5
cycles/row, 256-deep contraction per instruction) and fp32 PSUM
accumulation. Error budget analysis (numsim.py) showed the output error
is dominated by a rectification bias from fp8 CONV-head weight rounding
(relu/maxpool turn zero-mean weight noise into a positive output shift),
not by the GGNN fp8 passes -- so the e5m2 residual correction lives on
the conv weights (cheap: once per graph) instead of on the GGNN weights
(expensive: every step), cutting per-block PE work from 136 to 92 matmul
instructions.

Layouts are transposed host-side so the hidden dim lives on SBUF
partitions: h^T is [256(=2x128 chunks), 2048 nodes] per core.
"""

import sys

if "/opt/trn_rl_repo" not in sys.path:
    sys.path.insert(0, "/opt/trn_rl_repo")

import numpy as np
import ml_dtypes

B, NPG, HID, NET, E, STEPS = 32, 512, 256, 13, 262144, 6
NCORES = 8
GPC = B // NCORES          # graphs per core = 4
NLOC = GPC * NPG           # local nodes = 2048
EPG = E // B               # edges per graph = 8192

_CACHE = {}


def _weighted_pattern(weights, n):
    """Interleaved engine assignment: n slots split per weights dict."""
    weights = {k: w for k, w in weights.items() if w > 0}
    credit = {k: 0.0 for k in weights}
    total = float(sum(weights.values()))
    out = []
    for _ in range(n):
        for k, w in weights.items():
            credit[k] += w / total
        pick = max(credit, key=lambda k: credit[k])
        credit[pick] -= 1.0
        out.append(pick)
    return out


def _build_nc(steps=None, conv=None, skew=5,
              copy_w=(8, 6, 0)):
    steps = STEPS if steps is None else steps
    conv = True if conv is None else conv
    import concourse.bass as bass  # noqa: F401
    import concourse.tile as tile
    from concourse import mybir, bacc
    from contextlib import ExitStack

    f32 = mybir.dt.float32
    bf16 = mybir.dt.bfloat16
    f8 = mybir.dt.float8e4
    f85 = mybir.dt.float8e5
    AF = mybir.ActivationFunctionType
    ALU = mybir.AluOpType
    DR = mybir.MatmulPerfMode.DoubleRow

    nc = bacc.Bacc(None, target_bir_lowering=False)

    xTf_d = nc.dram_tensor("xTf", [2, 128, NLOC], f32, kind="ExternalInput")
    xT8_d = nc.dram_tensor("xT8", [2, 128, NLOC], f8, kind="ExternalInput")
    A8_d = nc.dram_tensor("A8", [GPC, NET, 4, 128, NPG], f8, kind="ExternalInput")
    We_d = nc.dram_tensor("We8", [2, 128, NET, HID], f8, kind="ExternalInput")
    wih_d = nc.dram_tensor("wih8T", [2, 128, 3 * HID], f8, kind="ExternalInput")
    whh_d = nc.dram_tensor("whh8T", [2, 128, 3 * HID], f8, kind="ExternalInput")
    rzb_d = nc.dram_tensor("rzb", [128, 4], f32, kind="ExternalInput")
    ginb_d = nc.dram_tensor("ginb", [128, 2], f32, kind="ExternalInput")
    ghnb_d = nc.dram_tensor("ghnb", [128, 2], f32, kind="ExternalInput")
    C8_d = nc.dram_tensor("C8", [GPC, 8, 2, NPG], f8, kind="ExternalInput")
    U8_d = nc.dram_tensor("U8", [8, 2, HID], f8, kind="ExternalInput")
    w1_d = nc.dram_tensor("w1D", [3, 128, 2, HID], f8, kind="ExternalInput")
    w1r_d = nc.dram_tensor("w1r", [3, 128, 2, HID], f85, kind="ExternalInput")
    b1_d = nc.dram_tensor("b1", [128, 2], f32, kind="ExternalInput")
    w2_d = nc.dram_tensor("w2T8", [2, 128, HID], f8, kind="ExternalInput")
    w2r_d = nc.dram_tensor("w2r", [2, 128, HID], f85, kind="ExternalInput")
    b2_d = nc.dram_tensor("b2", [128, 2], f32, kind="ExternalInput")
    wc1_d = nc.dram_tensor("wc1D", [3, 128, 2, 2, 2 * HID], f8, kind="ExternalInput")
    wc1r_d = nc.dram_tensor("wc1r", [3, 128, 2, 2, 2 * HID], f85, kind="ExternalInput")
    bc1_d = nc.dram_tensor("bc1", [128, 4], f32, kind="ExternalInput")
    wc2_d = nc.dram_tensor("wc2T8", [4, 128, 2 * HID], f8, kind="ExternalInput")
    wc2r_d = nc.dram_tensor("wc2r", [4, 128, 2 * HID], f85, kind="ExternalInput")
    bc2_d = nc.dram_tensor("bc2", [128, 4], f32, kind="ExternalInput")
    wy_d = nc.dram_tensor("wy", [128, 2], bf16, kind="ExternalInput")
    wz_d = nc.dram_tensor("wz", [128, 4], bf16, kind="ExternalInput")
    byz_d = nc.dram_tensor("byz", [1, 2], f32, kind="ExternalInput")
    out_d = nc.dram_tensor("out", [1, GPC], f32, kind="ExternalOutput")

    with tile.TileContext(nc) as tc, ExitStack() as top:
        state = top.enter_context(tc.tile_pool(name="state", bufs=1))
        h_t = state.tile([128, 2, NLOC], f32)     # h master state, hid-major
        h8_t = state.tile([128, 2, NLOC], f8)     # fp8 shadow of h for the PE

        cc = top.enter_context(tc.tile_pool(name="cc", bufs=1))
        w1_t = cc.tile([128, 3, 2, HID], f8)
        w1r_t = cc.tile([128, 3, 2, HID], f85)
        b1_t = cc.tile([128, 2], f32)
        w2_t = cc.tile([128, 2, HID], f8)
        w2r_t = cc.tile([128, 2, HID], f85)
        b2_t = cc.tile([128, 2], f32)
        wc1_t = cc.tile([128, 3, 2, 2, 2 * HID], f8)
        wc1r_t = cc.tile([128, 3, 2, 2, 2 * HID], f85)
        bc1_t = cc.tile([128, 4], f32)
        wc2_t = cc.tile([128, 4, 2 * HID], f8)
        wc2r_t = cc.tile([128, 4, 2 * HID], f85)
        bc2_t = cc.tile([128, 4], f32)
        wy_t = cc.tile([128, 2], bf16)
        wz_t = cc.tile([128, 4], bf16)
        byz_t = cc.tile([1, 2], f32)
        xD_t = cc.tile([128, 2, NLOC], f8)

        WS = 0.125   # weights shipped as 8x in fp8 (avoids subnormals)

        def copy_on(eng, dst, src):
            # psum -> sbuf fp8 eviction (the 1/8 fold lives in A8 = counts/8)
            if eng == "A":
                nc.scalar.copy(dst, src)
            elif eng == "D":
                nc.vector.tensor_copy(dst, src)
            else:
                nc.gpsimd.tensor_copy(dst, src)

        # ---------------- GGNN: 6 message-passing + GRU steps ----------------
        with ExitStack() as gg:
            cg = gg.enter_context(tc.tile_pool(name="cg", bufs=1))
            We_t = cg.tile([128, 2, NET, HID], f8)
            for k in range(2):
                nc.sync.dma_start(
                    We_t[:, :, k, :], We_d[:, :, k, :].rearrange("c p h -> p c h")
                )
            # graph 0's fp8 h shadow in half-graph chunks: the first stage-1
            # unit only needs nodes 0:256, so compute starts almost at once
            for half in range(2):
                hsl = slice(half * 256, (half + 1) * 256)
                for kc in range(2):
                    nc.sync.dma_start(h8_t[:, kc, hsl], xT8_d[kc, :, hsl])
            C8_t = cg.tile([8, GPC, 2, NPG], f8)
            U8_t = cg.tile([8, 2, HID], f8)
            nc.sync.dma_start(U8_t[:], U8_d[:])
            Ap = gg.enter_context(tc.tile_pool(name="Ap", bufs=1))
            A_t = Ap.tile([128, GPC, NET, 4, NPG], f8)
            for k in range(NET):
                if k >= 2:
                    nc.sync.dma_start(
                        We_t[:, :, k, :], We_d[:, :, k, :].rearrange("c p h -> p c h")
                    )
                # A loads split across the two HWDGE queues (SP + Act)
                (nc.sync if k % 2 == 0 else nc.scalar).dma_start(
                    A_t[:, 0, k, :, :], A8_d[0, k].rearrange("m p d -> p m d")
                )
            # priority order: everything block (0,g) needs lands just
            # before the A matrices it will consume
            def g_small(g):
                gsl = slice(g * NPG, (g + 1) * NPG)
                nc.sync.dma_start(C8_t[:, g, :, :], C8_d[g])
                for kc in range(2):
                    nc.sync.dma_start(h_t[:, kc, gsl], xTf_d[kc, :, gsl])

            for kc in range(2):
                nc.sync.dma_start(h8_t[:, kc, NPG:2 * NPG], xT8_d[kc, :, NPG:2 * NPG])
            g_small(0)
            wih_t = cg.tile([128, 2, 3 * HID], f8)
            nc.sync.dma_start(wih_t[:], wih_d.rearrange("c p m -> p c m"))
            whh_t = cg.tile([128, 2, 3 * HID], f8)
            nc.sync.dma_start(whh_t[:], whh_d.rearrange("c p m -> p c m"))
            rzb_t = cg.tile([128, 4], f32)
            nc.sync.dma_start(rzb_t[:], rzb_d[:])
            ginb_t = cg.tile([128, 2], f32)
            nc.sync.dma_start(ginb_t[:], ginb_d[:])
            ghnb_t = cg.tile([128, 2], f32)
            nc.sync.dma_start(ghnb_t[:], ghnb_d[:])
            for g in range(1, GPC):
                gsl = slice(g * NPG, (g + 1) * NPG)
                for k in range(NET):
                    (nc.sync if k % 2 == 0 else nc.scalar).dma_start(
                        A_t[:, g, k, :, :], A8_d[g, k].rearrange("m p d -> p m d")
                    )
                if g < GPC - 1:
                    ns = slice((g + 1) * NPG, (g + 2) * NPG)
                    for kc in range(2):
                        nc.sync.dma_start(h8_t[:, kc, ns], xT8_d[kc, :, ns])
                g_small(g)
            for kc in range(2):
                nc.sync.dma_start(xD_t[:, kc, :], xT8_d[kc, :, :])

            nc.sync.dma_start(w1_t[:], w1_d.rearrange("t p c o -> p t c o"))
            nc.sync.dma_start(w1r_t[:], w1r_d.rearrange("t p c o -> p t c o"))
            nc.sync.dma_start(b1_t[:], b1_d[:])
            nc.sync.dma_start(w2_t[:], w2_d.rearrange("c p o -> p c o"))
            nc.sync.dma_start(w2r_t[:], w2r_d.rearrange("c p o -> p c o"))
            nc.sync.dma_start(b2_t[:], b2_d[:])
            nc.sync.dma_start(wc1_t[:], wc1_d.rearrange("t p a b o -> p t a b o"))
            nc.sync.dma_start(wc1r_t[:], wc1r_d.rearrange("t p a b o -> p t a b o"))
            nc.sync.dma_start(bc1_t[:], bc1_d[:])
            nc.sync.dma_start(wc2_t[:], wc2_d.rearrange("c p o -> p c o"))
            nc.sync.dma_start(wc2r_t[:], wc2r_d.rearrange("c p o -> p c o"))
            nc.sync.dma_start(bc2_t[:], bc2_d[:])
            nc.sync.dma_start(wy_t[:], wy_d[:])
            nc.sync.dma_start(wz_t[:], wz_d[:])
            nc.sync.dma_start(byz_t[:], byz_d[:])

            # PSUM budget (8 banks): one shared ring of [128,2,512] pair
            # tiles (bufs=3, 6 banks) serves s1 units, gate pairs and conv
            # chains; aT [128,2,512] (2 banks) holds the block accumulator.
            ps_s1 = top.enter_context(tc.tile_pool(name="ps_s1", bufs=3, space="PSUM"))
            ps_aT = top.enter_context(tc.tile_pool(name="ps_aT", bufs=1, space="PSUM"))
            tn_p = gg.enter_context(tc.tile_pool(name="tn", bufs=skew + 3))
            wk = gg.enter_context(tc.tile_pool(name="wk", bufs=2))
            wk1 = gg.enter_context(tc.tile_pool(name="wk1", bufs=1))

            # ---- global software pipeline over all (step, graph) blocks ----
            # Stage-2 DoubleRow matmuls for unit i are emitted SKEW units
            # later; the pipeline runs straight through block boundaries.
            SKEW = skew
            kgroups = [(2 * q, min(2, NET - 2 * q)) for q in range((NET + 1) // 2)]
            units = [(k0, nk, pi) for (k0, nk) in kgroups for pi in range(2)]
            NU = len(units)
            blocks = [(s, g) for s in range(steps) for g in range(GPC)]
            pend = []  # (tnD, s, g, k0, nk, pi, idx_in_block)
            aT_of = {}  # g -> live aT_ps tile

            # psum-eviction copy engines, interleaved A(ct)/D(ve): one merged
            # [128, 2, w] eviction per unit (both node-halves at once)
            cpat = _weighted_pattern(
                {"A": copy_w[0], "D": copy_w[1], "P": copy_w[2]}, NU
            )
            tp_of = {}

            def emit_s1_half(s, g, k0, nk, pi, j, tnD, cslot):
                w = nk * HID
                m = g * 4 + 2 * pi + j
                msl = slice(m * 128, (m + 1) * 128)
                if j == 0:
                    tp_of[id(tnD)] = ps_s1.tile(
                        [128, 2, 2 * HID], f32, name="tn_ps", tag="s1"
                    )
                tp = tp_of[id(tnD)]
                nc.tensor.matmul(
                    tp[:, j, :w], h8_t[:, :, msl], We_t[:, :, k0:k0 + nk, :],
                    start=True, stop=True, perf_mode=DR,
                )
                if j == 1:
                    tp = tp_of.pop(id(tnD))
                    copy_on(cpat[cslot // 2], tnD[:, :, :w], tp[:, :, :w])

            def emit_s2_half(item, hc):
                tnD, s, g, k0, nk, pi, idx = item
                if idx == 0 and hc == 0:
                    aT_of[g] = ps_aT.tile([128, 2, NPG], f32, name="aT_ps", tag="aT_ps")
                aT_ps = aT_of[g]
                if idx == 1:
                    # per-node bias term: aT += b_e^T @ C (rank-13 as fp8 DR)
                    nc.tensor.matmul(
                        aT_ps[:, hc, :],
                        U8_t[:, :, hc * 128:(hc + 1) * 128],
                        C8_t[:, g, :, :],
                        start=False, stop=False, perf_mode=DR,
                    )
                for ko in range(nk):
                    nc.tensor.matmul(
                        aT_ps[:, hc, :],
                        tnD[:, :, ko * HID + hc * 128:ko * HID + (hc + 1) * 128],
                        A_t[:, g, k0 + ko, 2 * pi:2 * pi + 2, :],
                        start=(idx == 0 and ko == 0),
                        stop=(idx == NU - 1 and ko == nk - 1),
                        perf_mode=DR,
                    )

            # fine-grained deferral queue: (due_tick, seq, closure). Gate-tail
            # ops are drizzled into the next block's unit stream so each op's
            # deps are complete before it reaches its engine queue head (no
            # head-of-line blocking of the psum-eviction copies).
            import heapq

            defer_q = []
            dseq = [0]
            tick = 0

            def defer(dt, fn):
                dseq[0] += 1
                heapq.heappush(defer_q, (tick + dt, dseq[0], fn))

            def emit_gru(s, g):
                gsl = slice(g * NPG, (g + 1) * NPG)
                aT_ps = aT_of.pop(g)
                # aggregated messages (bias already accumulated) -> fp8
                aT8 = wk.tile([128, 2, NPG], f8, tag="aT")
                if g % 2 == 0:
                    nc.scalar.copy(aT8[:], aT_ps[:])
                else:
                    nc.vector.tensor_copy(aT8[:], aT_ps[:])
                r_t = wk1.tile([128, 2, NPG], bf16, tag="r")
                z_t = wk1.tile([128, 2, NPG], bf16, tag="z")
                t1_t = wk1.tile([128, 2, NPG], bf16, tag="t1")
                n_t = wk1.tile([128, 2, NPG], bf16, tag="n")
                # whh-side matmuls first (depend only on h8, ready early);
                # the wih-side accumulates on top once aT8 lands.
                gate_ps = {}

                def rz_whh(ja):
                    # gates (2ja, 2ja+1) share one pair tile from the ring
                    tl = ps_s1.tile([128, 2, 2 * HID], f32, name="grz", tag="s1")
                    gate_ps[ja] = tl
                    for j in range(2):
                        jc = 2 * ja + j
                        csl = slice(jc * 128, (jc + 1) * 128)
                        nc.tensor.matmul(
                            tl[:, j, :NPG], whh_t[:, :, csl], h8_t[:, :, gsl],
                            start=True, stop=False, perf_mode=DR,
                        )

                def rz_wih(ja):
                    tl = gate_ps.pop(ja)
                    for j in range(2):
                        jc = 2 * ja + j
                        csl = slice(jc * 128, (jc + 1) * 128)
                        nc.tensor.matmul(
                            tl[:, j, :NPG], wih_t[:, :, csl], aT8[:],
                            start=False, stop=True, perf_mode=DR,
                        )
                        dst = r_t if jc < 2 else z_t
                        nc.scalar.activation(
                            dst[:, jc % 2, :], tl[:, j, :NPG], AF.Sigmoid,
                            bias=rzb_t[:, jc:jc + 1], scale=WS,
                        )

                rz_whh(0)
                defer(0.5, lambda: (rz_wih(0), rz_whh(1)))
                defer(1.0, lambda: rz_wih(1))
                # n-gate: t1 = (gh_n + bhh_n)*r + gi_n; n = tanh(t1 + bih_n)
                # h' = n + z*(h - n)
                nps = {}

                def n_mm(hc):
                    csl = slice(512 + hc * 128, 512 + (hc + 1) * 128)
                    tl = ps_s1.tile([128, 2, 2 * HID], f32, name="pgpi", tag="s1")
                    nc.tensor.matmul(
                        tl[:, 0, :NPG], whh_t[:, :, csl], h8_t[:, :, gsl],
                        start=True, stop=True, perf_mode=DR,
                    )
                    nc.tensor.matmul(
                        tl[:, 1, :NPG], wih_t[:, :, csl], aT8[:],
                        start=True, stop=True, perf_mode=DR,
                    )
                    nps[hc] = tl

                def n_stt(hc):
                    tl = nps[hc]
                    nc.vector.scalar_tensor_tensor(
                        t1_t[:, hc, :], tl[:, 0, :NPG], ghnb_t[:, hc:hc + 1],
                        r_t[:, hc, :], op0=ALU.add, op1=ALU.mult,
                    )

                def n_add(hc):
                    # fold the 8x bih_n bias in here so one bias-free tanh
                    # can cover both hid chunks in a single Act op
                    tl = nps.pop(hc)
                    nc.vector.scalar_tensor_tensor(
                        t1_t[:, hc, :], tl[:, 1, :NPG], ginb_t[:, hc:hc + 1],
                        t1_t[:, hc, :], op0=ALU.add, op1=ALU.add,
                    )

                def n_tanh():
                    nc.scalar.activation(
                        n_t[:], t1_t[:], AF.Tanh, scale=WS,
                    )

                def upd_a(hc):
                    nc.gpsimd.tensor_sub(
                        t1_t[:, hc, :], h_t[:, hc, gsl], n_t[:, hc, :]
                    )
                    nc.gpsimd.tensor_mul(
                        t1_t[:, hc, :], t1_t[:, hc, :], z_t[:, hc, :]
                    )

                def upd_b(hc):
                    nc.gpsimd.tensor_add(
                        h_t[:, hc, gsl], n_t[:, hc, :], t1_t[:, hc, :]
                    )

                def upd_c(hc):
                    nc.gpsimd.tensor_copy(h8_t[:, hc, gsl], h_t[:, hc, gsl])

                defer(1.5, lambda: n_mm(0))
                defer(2.0, lambda: n_stt(0))
                defer(2.5, lambda: (n_add(0), n_mm(1)))
                defer(3.0, lambda: n_stt(1))
                defer(3.5, lambda: n_add(1))
                defer(4.0, n_tanh)
                defer(4.5, lambda: upd_a(0))
                defer(5.0, lambda: (upd_b(0), upd_a(1)))
                defer(5.5, lambda: (upd_c(0), upd_b(1)))
                defer(6.0, lambda: upd_c(1))

            def pop_tail(item):
                _, ps, pg, _, _, _, pidx = item
                if pidx == NU - 1:
                    emit_gru(ps, pg)

            def drain_due():
                while defer_q and defer_q[0][0] <= tick:
                    heapq.heappop(defer_q)[2]()

            for (s, g) in blocks:
                for idx, (k0, nk, pi) in enumerate(units):
                    tick += 0.5
                    tnD = tn_p.tile([128, 2, 2 * HID], f8)
                    pend.append((tnD, s, g, k0, nk, pi, idx))
                    emit_s1_half(s, g, k0, nk, pi, 0, tnD, 2 * idx)
                    if len(pend) > SKEW:
                        emit_s2_half(pend[0], 0)
                    drain_due()
                    tick += 0.5
                    emit_s1_half(s, g, k0, nk, pi, 1, tnD, 2 * idx + 1)
                    if len(pend) > SKEW:
                        item = pend.pop(0)
                        emit_s2_half(item, 1)
                        pop_tail(item)
                    drain_due()
            while pend:
                item = pend.pop(0)
                tick += 1
                emit_s2_half(item, 0)
                emit_s2_half(item, 1)
                pop_tail(item)
                drain_due()
            while defer_q:
                heapq.heappop(defer_q)[2]()

        # ---------------- conv head + readout ----------------
        # Convs accumulate in PSUM with NO bias/relu; maxpool (monotone)
        # commutes with relu(x+b), so pools read PSUM directly and the
        # Act engine fuses bias+relu on the pooled output. Each conv
        # weight has an e5m2 residual pass (fp8+res ~= bf16 weights):
        # the relu/maxpool rectification bias from fp8 weight rounding
        # was the dominant output error.
        with ExitStack() as cv:
          if conv:
              outp = cv.enter_context(tc.tile_pool(name="outp", bufs=1))
              out_sb = outp.tile([1, GPC], f32)
              cw = cv.enter_context(tc.tile_pool(name="cw", bufs=2))
              L1, L2, L3 = 510, 254, 127

              zchain = {}

              def ps_pair(co, ln):
                  # chains pair up inside the 2-bank ring tiles
                  if co % 2 == 0:
                      zchain[0] = ps_s1.tile(
                          [128, 2, 2 * HID], f32, name="cvz", tag="s1"
                      )
                  return zchain[0][:, co % 2, :ln]

              def relu_evac(i, dst, src, bias):
                  # psum -> sbuf relu(x + b), alternating Act / DVE
                  if i % 2 == 0:
                      nc.scalar.activation(dst, src, AF.Relu, bias=bias, scale=1.0)
                  else:
                      nc.vector.tensor_scalar(
                          dst, src, bias, 0.0, op0=ALU.add, op1=ALU.max
                      )

              def pools3(i, dst, src):
                  # maxpool k=3 s=2 on sbuf bf16 (DVE; Pool has no max ALU)
                  e = src.rearrange("p (l s) -> p l s", s=2)
                  Lo = e.shape[1] - 1
                  nc.vector.tensor_max(dst, e[:, :Lo, 0], e[:, :Lo, 1])
                  nc.vector.tensor_max(dst, dst, e[:, 1:Lo + 1, 0])

              def pools2(i, dst, src):
                  e = src.rearrange("p (l s) -> p l s", s=2)
                  nc.vector.tensor_max(dst, e[:, :, 0], e[:, :, 1])

              # Phase A: stage-1 convs (K=3) + relu-evac + pools, all graphs.
              y1s, z1s = [], []
              for g in range(GPC):
                  gof = g * NPG
                  y1 = cw.tile([128, 2, L2], f8, tag="y1", bufs=4)
                  y1p = cw.tile([128, 2, L1], bf16, tag="y1p")
                  for co in range(2):
                      p_ = ps_pair(co, L1)
                      for t in range(3):
                          nc.tensor.matmul(
                              p_,
                              w1_t[:, t, :, co * 128:(co + 1) * 128],
                              h8_t[:, :, gof + t:gof + t + L1],
                              start=(t == 0), stop=False,
                              perf_mode=DR,
                          )
                      for t in range(3):
                          nc.tensor.matmul(
                              p_,
                              w1r_t[:, t, :, co * 128:(co + 1) * 128],
                              h8_t[:, :, gof + t:gof + t + L1],
                              start=False, stop=(t == 2),
                              perf_mode=DR,
                          )
                      relu_evac(co, y1p[:, co, :], p_, b1_t[:, co:co + 1])
                      pools3(co, y1[:, co, :], y1p[:, co, :])
                  z1 = cw.tile([128, 4, L2], f8, tag="z1", bufs=4)
                  z1p = cw.tile([128, 4, L1], bf16, tag="z1p")
                  for co in range(4):
                      p_ = ps_pair(co, L1)
                      idx = 0
                      for wt, res in ((wc1_t, False), (wc1r_t, True)):
                          for t in range(3):
                              for pr in range(2):
                                  rhs = (h8_t if pr == 0 else xD_t)[
                                      :, :, gof + t:gof + t + L1
                                  ]
                                  nc.tensor.matmul(
                                      p_,
                                      wt[:, t, pr, :, co * 128:(co + 1) * 128],
                                      rhs,
                                      start=(idx == 0), stop=(idx == 11),
                                      perf_mode=DR,
                                  )
                                  idx += 1
                      relu_evac(co, z1p[:, co, :], p_, bc1_t[:, co:co + 1])
                      pools3(co + 1, z1[:, co, :], z1p[:, co, :])
                  y1s.append(y1)
                  z1s.append(z1)

              # Phase B: K=1 convs (DoubleRow) + relu-evac + final pools.
              y2s, z2s = [], []
              for g in range(GPC):
                  y1, z1 = y1s[g], z1s[g]
                  y2 = cw.tile([128, 2, L3], bf16, tag="y2", bufs=4)
                  y2p = cw.tile([128, 2, L2], bf16, tag="y2p")
                  for co in range(2):
                      p_ = ps_pair(co, L2)
                      nc.tensor.matmul(
                          p_, w2_t[:, :, co * 128:(co + 1) * 128], y1[:],
                          start=True, stop=False, perf_mode=DR,
                      )
                      nc.tensor.matmul(
                          p_, w2r_t[:, :, co * 128:(co + 1) * 128], y1[:],
                          start=False, stop=True, perf_mode=DR,
                      )
                      relu_evac(co, y2p[:, co, :], p_, b2_t[:, co:co + 1])
                      pools2(co, y2[:, co, :], y2p[:, co, :])
                  z2 = cw.tile([128, 4, L3], bf16, tag="z2", bufs=4)
                  z2p = cw.tile([128, 4, L2], bf16, tag="z2p")
                  for co in range(4):
                      p_ = ps_pair(co, L2)
                      for wt, last in ((wc2_t, False), (wc2r_t, True)):
                          for t in range(2):
                              nc.tensor.matmul(
                                  p_,
                                  wt[:, 2 * t:2 * t + 2, co * 128:(co + 1) * 128],
                                  z1[:, 2 * t:2 * t + 2, :],
                                  start=(not last and t == 0),
                                  stop=(last and t == 1),
                                  perf_mode=DR,
                              )
                      relu_evac(co, z2p[:, co, :], p_, bc2_t[:, co:co + 1])
                      pools2(co + 1, z2[:, co, :], z2p[:, co, :])
                  y2s.append(y2)
                  z2s.append(z2)

              # Phase B2: readouts, covered by each other's matmuls
              for g in range(GPC):
                  y2, z2 = y2s[g], z2s[g]
                  # readout: sigmoid(mean((Y2 wy+by)*(Z2 wz+bz)))
                  ro = ps_s1.tile([128, 2, 2 * HID], f32, name="ro", tag="s1")
                  zp = ro[:1, 0, :L3]
                  yp = ro[:1, 1, :L3]
                  for ci in range(4):
                      nc.tensor.matmul(
                          zp, wz_t[:, ci:ci + 1], z2[:, ci, :],
                          start=(ci == 0), stop=(ci == 3),
                      )
                  zb = cw.tile([1, L3], f32, tag="zb")
                  nc.vector.tensor_scalar_add(zb[:], zp, byz_t[:1, 1:2])
                  for hc in range(2):
                      nc.tensor.matmul(
                          yp, wy_t[:, hc:hc + 1], y2[:, hc, :],
                          start=(hc == 0), stop=(hc == 1),
                      )
                  # (yp+by)*zb with the row-sum fused via accum_out
                  yb = cw.tile([1, L3], f32, tag="yb")
                  sacc = cw.tile([1, 1], f32, tag="sacc")
                  nc.vector.scalar_tensor_tensor(
                      yb[:], yp, byz_t[:1, 0:1], zb[:],
                      op0=ALU.add, op1=ALU.mult,
                      accum_out=sacc[:],
                  )
                  nc.scalar.activation(
                      out_sb[:1, g:g + 1], sacc[:], AF.Sigmoid, scale=1.0 / L3
                  )
              nc.sync.dma_start(out_d[:], out_sb[:])

    nc.compile()
    return nc


def _host_prep(inputs):
    """Full inputs -> list of 8 per-core input dicts."""
    bf16 = ml_dtypes.bfloat16
    f8 = ml_dtypes.float8_e4m3
    f85 = ml_dtypes.float8_e5m2

    x = np.asarray(inputs["x"], np.float32)
    src = np.asarray(inputs["src"], np.int32)
    dst = np.asarray(inputs["dst"], np.int32)
    et = np.asarray(inputs["etype"], np.int32)
    W_e = np.asarray(inputs["W_e"], np.float32)
    b_e = np.asarray(inputs["b_e"], np.float32)
    wih = np.asarray(inputs["gru_wih"], np.float32)
    whh = np.asarray(inputs["gru_whh"], np.float32)
    bih = np.asarray(inputs["gru_bih"], np.float32)
    bhh = np.asarray(inputs["gru_bhh"], np.float32)

    def res_pair(a):
        q = a.astype(f8)
        return q, (a - q.astype(np.float32)).astype(f85)

    We8q = np.ascontiguousarray(
        np.transpose(W_e.reshape(NET, 2, 128, HID), (1, 2, 0, 3)) * 8.0
    ).astype(f8)
    wih8q = np.ascontiguousarray(wih.T.reshape(2, 128, 3 * HID) * 8.0).astype(f8)
    whh8q = np.ascontiguousarray(whh.T.reshape(2, 128, 3 * HID) * 8.0).astype(f8)
    U = np.zeros((16, HID), np.float32)
    U[:NET] = b_e

    # conv weights ship UNSCALED (x1): the e5m2 residual pass absorbs any
    # fp8 subnormal loss, and scale-free psums let relu-evac run as a DVE
    # tensor_scalar (engine-movable) instead of an Act-only activation.
    w1a = np.ascontiguousarray(
        np.transpose(
            np.transpose(np.asarray(inputs["conv1_w"], np.float32), (2, 1, 0))
            .reshape(3, 2, 128, HID), (0, 2, 1, 3)
        )
    )
    w1q, w1r = res_pair(w1a)
    w2a = np.ascontiguousarray(
        np.asarray(inputs["conv2_w"], np.float32)[:, :, 0].T.reshape(2, 128, HID)
    )
    w2q, w2r = res_pair(w2a)
    wc1a = np.ascontiguousarray(
        np.transpose(
            np.transpose(np.asarray(inputs["cconv1_w"], np.float32), (2, 1, 0))
            .reshape(3, 2, 2, 128, 2 * HID), (0, 3, 1, 2, 4)
        )
    )
    wc1q, wc1r = res_pair(wc1a)
    wc2a = np.ascontiguousarray(
        np.asarray(inputs["cconv2_w"], np.float32)[:, :, 0].T.reshape(
            4, 128, 2 * HID
        )
    )
    wc2q, wc2r = res_pair(wc2a)

    shared = {
        "U8": np.ascontiguousarray(
            U.reshape(2, 8, HID).transpose(1, 0, 2) * 8.0
        ).astype(f8),
        "We8": We8q,
        "wih8T": wih8q,
        "whh8T": whh8q,
        "rzb": np.ascontiguousarray((bih + bhh)[: 2 * HID].reshape(4, 128).T).astype(
            np.float32
        ),
        "ginb": np.ascontiguousarray(bih[2 * HID:].reshape(2, 128).T * 8.0).astype(np.float32),
        "ghnb": np.ascontiguousarray(bhh[2 * HID:].reshape(2, 128).T * 8.0).astype(np.float32),
        "w1D": w1q,
        "w1r": w1r,
        "b1": np.ascontiguousarray(
            np.asarray(inputs["conv1_b"], np.float32).reshape(2, 128).T
        ),
        "w2T8": w2q,
        "w2r": w2r,
        "b2": np.ascontiguousarray(
            np.asarray(inputs["conv2_b"], np.float32).reshape(2, 128).T
        ),
        "wc1D": wc1q,
        "wc1r": wc1r,
        "bc1": np.ascontiguousarray(
            np.asarray(inputs["cconv1_b"], np.float32).reshape(4, 128).T
        ),
        "wc2T8": wc2q,
        "wc2r": wc2r,
        "bc2": np.ascontiguousarray(
            np.asarray(inputs["cconv2_b"], np.float32).reshape(4, 128).T
        ),
        "wy": np.ascontiguousarray(
            np.asarray(inputs["wy"], np.float32).reshape(2, 128).T
        ).astype(bf16),
        "wz": np.ascontiguousarray(
            np.asarray(inputs["wz"], np.float32).reshape(4, 128).T
        ).astype(bf16),
        "byz": np.array(
            [[float(np.asarray(inputs["by"]).reshape(-1)[0]),
              float(np.asarray(inputs["bz"]).reshape(-1)[0])]],
            np.float32,
        ),
    }

    in_maps = []
    for c in range(NCORES):
        n0 = c * NLOC
        esl = slice(c * GPC * EPG, (c + 1) * GPC * EPG)
        s_l = src[esl] - n0          # local node ids 0..2047
        d_l = dst[esl] - n0
        k_l = et[esl]
        g_l = s_l // NPG             # local graph 0..3 (edges stay in-graph)
        sg = s_l % NPG
        dg = d_l % NPG
        flat = ((g_l.astype(np.int64) * NET + k_l) * NPG + sg) * NPG + dg
        # counts/8: the 1/8 weight-scale fold lives here (exact in fp8)
        A = (
            np.bincount(flat, minlength=GPC * NET * NPG * NPG).astype(np.float32)
            * 0.125
        ).astype(f8)
        A8 = A.reshape(GPC, NET, 4, 128, NPG)

        # per-(graph, etype) destination in-degree counts, exact in fp8
        cflat = (g_l.astype(np.int64) * NET + k_l) * NPG + dg
        C = np.bincount(cflat, minlength=GPC * NET * NPG).astype(np.float32)
        C8 = np.zeros((GPC, 16, NPG), np.float32)
        C8[:, :NET, :] = C.reshape(GPC, NET, NPG)
        C8c = np.ascontiguousarray(
            C8.reshape(GPC, 2, 8, NPG).transpose(0, 2, 1, 3) * 0.125
        ).astype(f8)

        xTc = np.ascontiguousarray(x[n0:n0 + NLOC].T.reshape(2, 128, NLOC))

        x8c = xTc.astype(f8)
        m = {
            "xTf": xTc,
            "xT8": x8c,
            "A8": A8,
            "C8": C8c,
        }
        m.update(shared)
        in_maps.append(m)
    return in_maps


def _get_nc():
    if "nc" not in _CACHE:
        _CACHE["nc"] = _build_nc()
    return _CACHE["nc"]


def run(inputs, trace=False):
    from concourse.bass_utils import run_bass_kernel_spmd

    nc = _get_nc()
    in_maps = _host_prep(inputs)
    res = run_bass_kernel_spmd(
        nc, in_maps, core_ids=list(range(NCORES)), trace=trace
    )
    out = np.concatenate(
        [np.asarray(res.results[c]["out"], np.float32).reshape(-1) for c in range(NCORES)]
    )
    return out, res


def kernel(**inputs):
    out, _ = run(inputs, trace=False)
    return out
